# revision 17
# baseline (speedup 1.0000x reference)
"""VQ codebook-lookup kernel for Trainium2 (8 NeuronCores, SPMD data-parallel).

Problem: z (65536, 512) f32, embeddings (2048, 512) f32.
  zn   = z / max(||z||, 1e-12)
  sim  = (zn @ E^T) / max(||zn|| * ||E_k||, 1e-8)
  idx  = argmax_k sim
  out_embeddings = E[idx]  (straight-through; numerically == gathered)
  commitment_loss = 0.25 * mean((zn - E[idx])^2)
  perplexity from index histogram

Ranking trick: argmax_k sim == argmax_k (z . Ehat_k) with Ehat = E/||E_k||
(row-positive scaling doesn't change per-row argmax), so the kernel matmuls
raw z against the pre-normalized codebook.

Precision: dots are computed with a 3-term bf16 split
    z = zh + zl (bf16 hi/lo),  Ehat = eh + el
    z . Ehat ~= zh.eh + zh.el + zl.eh   (lo.lo term ~2^-18 rel, dropped)
bf16 products are exact in fp32 accumulation, so this matches an fp32 GEMM
to ~1e-7 (verified: 0 argmax flips vs the fp32 reference on the real data),
while running the PE at bf16 speed (3 cyc/row vs 4 for native fp32).

Sharding: data-parallel over N. Each core handles 8192 rows with a
replicated codebook; host concatenates shards and reduces the scalar stats.
"""

from contextlib import ExitStack

import numpy as np

import concourse.bacc as bacc
import concourse.bass as bass
import concourse.tile as tile
from concourse import mybir
from concourse.masks import make_identity
from concourse.bass_utils import run_bass_kernel_spmd


def _ensure_ntff_hook():
    """The agent image's antenv lacks axon_hooks; synthesize it so
    run_bass_kernel_spmd(trace=True) can NTFF-profile via libaxon_pjrt."""
    import sys
    import types
    try:
        import antenv.axon_hooks  # noqa: F401
        return
    except ImportError:
        pass
    try:
        import antenv
        from trn_agent_boot.trn_boot import _ntff_profile_via_ctypes
        mod = types.ModuleType("antenv.axon_hooks")
        hook = _ntff_profile_via_ctypes("/opt/axon/libaxon_pjrt.so")
        mod.get_axon_ntff_profile_hook = lambda: hook
        mod.set_axon_ntff_profile_hook = lambda h: None
        sys.modules["antenv.axon_hooks"] = mod
        antenv.axon_hooks = mod
    except Exception:
        pass
    try:
        import concourse.bass_utils as _bu
        _orig_upload = _bu.upload_artifacts

        def _safe_upload(tmpdir):
            try:
                return _orig_upload(tmpdir)
            except Exception:
                return tmpdir

        _bu.upload_artifacts = _safe_upload
    except Exception:
        pass

N_CORES = 8
N, K, D = 65536, 2048, 512
P = 128          # partitions
KC = 512         # sim free-dim chunk = one PSUM bank
N_DC = D // P    # 4 contraction chunks
N_KC = K // KC   # 4 sim chunks

F32 = mybir.dt.float32
BF16 = mybir.dt.bfloat16
U32 = mybir.dt.uint32

_CACHE = {}


def _build_nc(n_rows: int):
    """Build the per-core Bass program for n_rows of z (K, D fixed)."""
    nc = bacc.Bacc(trn_type="TRN2", target_bir_lowering=False, debug=False)

    z_t = nc.dram_tensor("z", [n_rows, D], F32, kind="ExternalInput")
    e_t = nc.dram_tensor("emb", [K, D], F32, kind="ExternalInput")
    oe_t = nc.dram_tensor("out_emb", [n_rows, D], F32, kind="ExternalOutput")
    oi_t = nc.dram_tensor("out_idx", [n_rows, 1], U32, kind="ExternalOutput")
    oc_t = nc.dram_tensor("out_csum", [n_rows // P, P, 1], F32, kind="ExternalOutput")

    z = z_t.ap()
    e = e_t.ap()
    oe = oe_t.ap()
    oi = oi_t.ap().rearrange("(t p) o -> t p o", p=P)
    oc = oc_t.ap()

    n_tiles = n_rows // P
    sub = mybir.AluOpType.subtract
    Square = mybir.ActivationFunctionType.Square
    Sqrt = mybir.ActivationFunctionType.Sqrt
    Copy = mybir.ActivationFunctionType.Copy

    with tile.TileContext(nc) as tc, ExitStack() as ctx:
        persist = ctx.enter_context(tc.tile_pool(name="persist", bufs=1))
        eprep = ctx.enter_context(tc.tile_pool(name="eprep", bufs=3))
        dstage = ctx.enter_context(tc.tile_pool(name="dstage", bufs=3, space="DRAM"))
        zpool = ctx.enter_context(tc.tile_pool(name="zp", bufs=3))
        tpool = ctx.enter_context(tc.tile_pool(name="zt", bufs=3))
        spool = ctx.enter_context(tc.tile_pool(name="sim", bufs=2))
        pspool = ctx.enter_context(tc.tile_pool(name="ps", bufs=5, space="PSUM"))
        tppool = ctx.enter_context(tc.tile_pool(name="tp", bufs=3, space="PSUM"))
        gpool = ctx.enter_context(tc.tile_pool(name="gp", bufs=3))
        small = ctx.enter_context(tc.tile_pool(name="small", bufs=4))

        ident = persist.tile([P, P], F32, tag="ident")
        make_identity(nc, ident[:])

        # ---- codebook prep: Ehat = E/||E_k||, bf16 hi/lo, transposed ----
        # staged via DRAM so the transposes use the DRAM->SBUF xbar path
        eh_dram = dstage.tile([K, D], BF16, tag="ehd")
        el_dram = dstage.tile([K, D], BF16, tag="eld")
        for kt in range(K // P):
            et = eprep.tile([P, D], F32, tag="et")
            nc.sync.dma_start(out=et[:], in_=e[kt * P:(kt + 1) * P, :])
            esq = eprep.tile([P, D], F32, tag="esq")
            esum = eprep.tile([P, 1], F32, tag="esum")
            nc.scalar.activation(esq[:], et[:], Square, accum_out=esum[:])
            enrm = eprep.tile([P, 1], F32, tag="enrm")
            nc.scalar.activation(enrm[:], esum[:], Sqrt)
            einv = eprep.tile([P, 1], F32, tag="einv")
            nc.vector.reciprocal(einv[:], enrm[:])
            ehat = eprep.tile([P, D], F32, tag="ehat")
            # ACT copy with per-partition scale (TensorScalarPtr on DVE has
            # too few HW sync-wait slots to take a DMA + engine wait)
            nc.scalar.activation(ehat[:], et[:], Copy, scale=einv[:])
            eh = eprep.tile([P, D], BF16, tag="eh")
            nc.scalar.activation(eh[:], ehat[:], Copy)
            el = eprep.tile([P, D], BF16, tag="el")
            nc.vector.tensor_tensor(el[:], ehat[:], eh[:], op=sub)
            nc.sync.dma_start(out=eh_dram[kt * P:(kt + 1) * P, :], in_=eh[:])
            nc.sync.dma_start(out=el_dram[kt * P:(kt + 1) * P, :], in_=el[:])

        # transposed codebook, persistent in SBUF: [d-chunk][128, K] bf16
        ehT = []
        elT = []
        for dc in range(N_DC):
            th = persist.tile([P, K], BF16, tag=f"ehT{dc}")
            nc.sync.dma_start_transpose(out=th[:], in_=eh_dram[:, dc * P:(dc + 1) * P])
            ehT.append(th)
            tl = persist.tile([P, K], BF16, tag=f"elT{dc}")
            nc.sync.dma_start_transpose(out=tl[:], in_=el_dram[:, dc * P:(dc + 1) * P])
            elT.append(tl)

        # ---- main loop over 128-row tiles (software-pipelined emission:
        # prep of tile t+1 is emitted before gemm/finalize of tile t so the
        # in-order engine queues never block next-tile prep behind
        # current-tile reductions) ----

        def prep(t):
            """Load z tile, row norms, fp32 PE-transpose + bf16 hi/lo split."""
            zt_ = zpool.tile([P, D], F32, tag="z", name=f"z{t}")
            nc.sync.dma_start(out=zt_[:], in_=z[t * P:(t + 1) * P, :])

            zsq = zpool.tile([P, D], F32, tag="zsq", name=f"zsq{t}")
            nsum = small.tile([P, 1], F32, tag="nsum", name=f"nsum{t}")
            nc.scalar.activation(zsq[:], zt_[:], Square, accum_out=nsum[:])
            nrm = small.tile([P, 1], F32, tag="nrm", name=f"nrm{t}")
            nc.scalar.activation(nrm[:], nsum[:], Sqrt)
            nrmc = small.tile([P, 1], F32, tag="nrmc", name=f"nrmc{t}")
            nc.vector.tensor_scalar_max(nrmc[:], nrm[:], 1e-12)
            inv = small.tile([P, 1], F32, tag="inv", name=f"inv{t}")
            nc.vector.reciprocal(inv[:], nrmc[:])

            # PE-transpose fp32 z chunks, then split hi/lo straight out of
            # PSUM (cast on ACT, exact residual on DVE)
            zhT = tpool.tile([P, N_DC, P], BF16, tag="zhT", name=f"zhT{t}")
            zlT = tpool.tile([P, N_DC, P], BF16, tag="zlT", name=f"zlT{t}")
            for dc in range(N_DC):
                tpf = tppool.tile([P, P], F32, tag="tp", name=f"tp{t}_{dc}")
                nc.tensor.transpose(tpf[:], zt_[:, dc * P:(dc + 1) * P], ident[:])
                nc.scalar.copy(out=zhT[:, dc, :], in_=tpf[:])
                nc.vector.tensor_tensor(
                    zlT[:, dc, :], tpf[:], zhT[:, dc, :], op=sub)
            return dict(z=zt_, inv=inv, zhT=zhT, zlT=zlT)

        def gemm(t, st):
            """48 accumulating matmuls -> sim [128, K] in SBUF."""
            zhT, zlT = st["zhT"], st["zlT"]
            sims = [pspool.tile([P, KC], F32, tag="sims", name=f"sims{t}_{kc}")
                    for kc in range(N_KC)]
            for dc in range(N_DC):
                for kc in range(N_KC):
                    ks = slice(kc * KC, (kc + 1) * KC)
                    nc.tensor.matmul(
                        sims[kc][:], zhT[:, dc, :], ehT[dc][:, ks],
                        start=(dc == 0), stop=False)
                    nc.tensor.matmul(
                        sims[kc][:], zhT[:, dc, :], elT[dc][:, ks],
                        start=False, stop=False)
            for dc in range(N_DC):
                for kc in range(N_KC):
                    ks = slice(kc * KC, (kc + 1) * KC)
                    nc.tensor.matmul(
                        sims[kc][:], zlT[:, dc, :], ehT[dc][:, ks],
                        start=False, stop=(dc == N_DC - 1))
            sim_sb = spool.tile([P, K], F32, tag="sim", name=f"sim{t}")
            for kc in range(N_KC):
                nc.scalar.copy(
                    out=sim_sb[:, kc * KC:(kc + 1) * KC], in_=sims[kc][:])
            return sim_sb

        def finalize(t, st, sim_sb):
            """argmax, gather, outputs, commitment partials."""
            rows = slice(t * P, (t + 1) * P)
            mx8 = small.tile([P, 8], F32, tag="mx8", name=f"mx8{t}")
            ix8 = small.tile([P, 8], U32, tag="ix8", name=f"ix8{t}")
            nc.vector.max(mx8[:], sim_sb[:])
            nc.vector.max_index(ix8[:], mx8[:], sim_sb[:])
            nc.sync.dma_start(out=oi[t, :, :], in_=ix8[:, 0:1])

            g = gpool.tile([P, D], F32, tag="g", name=f"g{t}")
            nc.gpsimd.indirect_dma_start(
                out=g[:], out_offset=None, in_=e,
                in_offset=bass.IndirectOffsetOnAxis(ap=ix8[:, 0:1], axis=0))
            nc.sync.dma_start(out=oe[rows, :], in_=g[:])

            zn = zpool.tile([P, D], F32, tag="zn", name=f"zn{t}")
            nc.scalar.activation(zn[:], st["z"][:], Copy, scale=st["inv"][:])
            diff = zpool.tile([P, D], F32, tag="diff", name=f"diff{t}")
            nc.vector.tensor_tensor(diff[:], zn[:], g[:], op=sub)
            dsq = zpool.tile([P, D], F32, tag="dsq", name=f"dsq{t}")
            csum = small.tile([P, 1], F32, tag="csum", name=f"csum{t}")
            nc.scalar.activation(dsq[:], diff[:], Square, accum_out=csum[:])
            nc.sync.dma_start(out=oc[t, :, :], in_=csum[:])

        state = {0: prep(0)}
        for t in range(n_tiles):
            if t + 1 < n_tiles:
                state[t + 1] = prep(t + 1)
            sim_sb = gemm(t, state[t])
            finalize(t, state[t], sim_sb)
            del state[t]

    nc.compile()
    return nc


def _get_nc(n_rows: int):
    if n_rows not in _CACHE:
        _CACHE[n_rows] = _build_nc(n_rows)
    return _CACHE[n_rows]


def kernel(z: np.ndarray, embeddings: np.ndarray, _trace: bool = False):
    z = np.ascontiguousarray(np.asarray(z, dtype=np.float32))
    emb = np.ascontiguousarray(np.asarray(embeddings, dtype=np.float32))
    assert z.shape == (N, D) and emb.shape == (K, D)
    ns = N // N_CORES

    nc = _get_nc(ns)
    in_maps = [
        {"z": z[c * ns:(c + 1) * ns], "emb": emb} for c in range(N_CORES)
    ]
    if _trace:
        _ensure_ntff_hook()
    res = run_bass_kernel_spmd(
        nc, in_maps, core_ids=list(range(N_CORES)), trace=_trace)

    out_emb = np.concatenate([r["out_emb"] for r in res.results], axis=0)
    idx = np.concatenate(
        [r["out_idx"].reshape(-1) for r in res.results]).astype(np.int32)
    csum = np.concatenate(
        [r["out_csum"].reshape(-1) for r in res.results])

    commitment_loss = np.float32(0.25 * (csum.astype(np.float64).sum() / (N * D)))
    counts = np.bincount(idx, minlength=K).astype(np.float64)
    avg = counts / N
    perplexity = np.float32(np.exp(-np.sum(avg * np.log(avg + 1e-10))))

    if _trace:
        kernel.last_exec_time_ns = res.exec_time_ns
        kernel.last_res = res
    return out_emb, idx, commitment_loss, perplexity


# revision 21
# speedup vs baseline: 1.0536x; 1.0536x over previous
"""VQ codebook-lookup kernel for Trainium2 (8 NeuronCores, SPMD data-parallel).

Problem: z (65536, 512) f32, embeddings (2048, 512) f32.
  zn   = z / max(||z||, 1e-12)
  sim  = (zn @ E^T) / max(||zn|| * ||E_k||, 1e-8)
  idx  = argmax_k sim
  out_embeddings = E[idx]  (straight-through; numerically == gathered)
  commitment_loss = 0.25 * mean((zn - E[idx])^2)
  perplexity from index histogram

Ranking trick: argmax_k sim == argmax_k (z . Ehat_k) with Ehat = E/||E_k||
(row-positive scaling doesn't change per-row argmax), so the kernel matmuls
raw z against the pre-normalized codebook.

Precision: dots are computed with a 3-term bf16 split
    z = zh + zl (bf16 hi/lo),  Ehat = eh + el
    z . Ehat ~= zh.eh + zh.el + zl.eh   (lo.lo term ~2^-18 rel, dropped)
bf16 products are exact in fp32 accumulation, so this matches an fp32 GEMM
to ~1e-7 (verified: 0 argmax flips vs the fp32 reference on the real data),
while running the PE at bf16 speed (3 cyc/row vs 4 for native fp32).

Sharding: data-parallel over N. Each core handles 8192 rows with a
replicated codebook; host concatenates shards and reduces the scalar stats.
"""

from contextlib import ExitStack

import numpy as np

import concourse.bacc as bacc
import concourse.bass as bass
import concourse.tile as tile
from concourse import mybir
from concourse.masks import make_identity
from concourse.bass_utils import run_bass_kernel_spmd


def _ensure_ntff_hook():
    """The agent image's antenv lacks axon_hooks; synthesize it so
    run_bass_kernel_spmd(trace=True) can NTFF-profile via libaxon_pjrt."""
    import sys
    import types
    try:
        import antenv.axon_hooks  # noqa: F401
        return
    except ImportError:
        pass
    try:
        import antenv
        from trn_agent_boot.trn_boot import _ntff_profile_via_ctypes
        mod = types.ModuleType("antenv.axon_hooks")
        hook = _ntff_profile_via_ctypes("/opt/axon/libaxon_pjrt.so")
        mod.get_axon_ntff_profile_hook = lambda: hook
        mod.set_axon_ntff_profile_hook = lambda h: None
        sys.modules["antenv.axon_hooks"] = mod
        antenv.axon_hooks = mod
    except Exception:
        pass
    try:
        import concourse.bass_utils as _bu
        _orig_upload = _bu.upload_artifacts

        def _safe_upload(tmpdir):
            try:
                return _orig_upload(tmpdir)
            except Exception:
                return tmpdir

        _bu.upload_artifacts = _safe_upload
    except Exception:
        pass

N_CORES = 8
N, K, D = 65536, 2048, 512
P = 128          # partitions
KC = 512         # sim free-dim chunk = one PSUM bank
N_DC = D // P    # 4 contraction chunks
N_KC = K // KC   # 4 sim chunks

F32 = mybir.dt.float32
BF16 = mybir.dt.bfloat16
U32 = mybir.dt.uint32

_CACHE = {}


def _build_nc(n_rows: int):
    """Build the per-core Bass program for n_rows of z (K, D fixed)."""
    nc = bacc.Bacc(trn_type="TRN2", target_bir_lowering=False, debug=False)

    z_t = nc.dram_tensor("z", [n_rows, D], F32, kind="ExternalInput")
    e_t = nc.dram_tensor("emb", [K, D], F32, kind="ExternalInput")
    oe_t = nc.dram_tensor("out_emb", [n_rows, D], F32, kind="ExternalOutput")
    oi_t = nc.dram_tensor("out_idx", [n_rows, 1], U32, kind="ExternalOutput")
    oc_t = nc.dram_tensor("out_csum", [n_rows // P, P, 1], F32, kind="ExternalOutput")

    z = z_t.ap()
    e = e_t.ap()
    oe = oe_t.ap()
    oi = oi_t.ap().rearrange("(t p) o -> t p o", p=P)
    oc = oc_t.ap()

    n_tiles = n_rows // P
    sub = mybir.AluOpType.subtract
    mul = mybir.AluOpType.mult
    add = mybir.AluOpType.add
    Square = mybir.ActivationFunctionType.Square
    Sqrt = mybir.ActivationFunctionType.Sqrt
    Copy = mybir.ActivationFunctionType.Copy

    with tile.TileContext(nc) as tc, ExitStack() as ctx:
        persist = ctx.enter_context(tc.tile_pool(name="persist", bufs=1))
        eprep = ctx.enter_context(tc.tile_pool(name="eprep", bufs=3))
        dstage = ctx.enter_context(tc.tile_pool(name="dstage", bufs=3, space="DRAM"))
        zpool = ctx.enter_context(tc.tile_pool(name="zp", bufs=3))
        tpool = ctx.enter_context(tc.tile_pool(name="zt", bufs=3))
        spool = ctx.enter_context(tc.tile_pool(name="sim", bufs=2))
        pspool = ctx.enter_context(tc.tile_pool(name="ps", bufs=5, space="PSUM"))
        tppool = ctx.enter_context(tc.tile_pool(name="tp", bufs=3, space="PSUM"))
        gpool = ctx.enter_context(tc.tile_pool(name="gp", bufs=3))
        small = ctx.enter_context(tc.tile_pool(name="small", bufs=4))

        ident = persist.tile([P, P], F32, tag="ident")
        make_identity(nc, ident[:])

        # ---- codebook prep: Ehat = E/||E_k||, bf16 hi/lo, transposed ----
        # staged via DRAM so the transposes use the DRAM->SBUF xbar path
        eh_dram = dstage.tile([K, D], BF16, tag="ehd")
        el_dram = dstage.tile([K, D], BF16, tag="eld")
        # per-code norms, kept in DRAM for the per-row norm gather
        enrm_dram = dstage.tile([K, 1], F32, tag="enrmd")
        for kt in range(K // P):
            et = eprep.tile([P, D], F32, tag="et")
            nc.sync.dma_start(out=et[:], in_=e[kt * P:(kt + 1) * P, :])
            esq = eprep.tile([P, D], F32, tag="esq")
            esum = eprep.tile([P, 1], F32, tag="esum")
            nc.scalar.activation(esq[:], et[:], Square, accum_out=esum[:])
            enrm = eprep.tile([P, 1], F32, tag="enrm")
            nc.scalar.activation(enrm[:], esum[:], Sqrt)
            nc.sync.dma_start(out=enrm_dram[kt * P:(kt + 1) * P, :], in_=enrm[:])
            einv = eprep.tile([P, 1], F32, tag="einv")
            nc.vector.reciprocal(einv[:], enrm[:])
            ehat = eprep.tile([P, D], F32, tag="ehat")
            # ACT copy with per-partition scale (TensorScalarPtr on DVE has
            # too few HW sync-wait slots to take a DMA + engine wait)
            nc.scalar.activation(ehat[:], et[:], Copy, scale=einv[:])
            eh = eprep.tile([P, D], BF16, tag="eh")
            nc.scalar.activation(eh[:], ehat[:], Copy)
            el = eprep.tile([P, D], BF16, tag="el")
            nc.vector.tensor_tensor(el[:], ehat[:], eh[:], op=sub)
            nc.sync.dma_start(out=eh_dram[kt * P:(kt + 1) * P, :], in_=eh[:])
            nc.sync.dma_start(out=el_dram[kt * P:(kt + 1) * P, :], in_=el[:])

        # transposed codebook, persistent in SBUF: [d-chunk][128, K] bf16
        ehT = []
        elT = []
        for dc in range(N_DC):
            th = persist.tile([P, K], BF16, tag=f"ehT{dc}")
            nc.sync.dma_start_transpose(out=th[:], in_=eh_dram[:, dc * P:(dc + 1) * P])
            ehT.append(th)
            tl = persist.tile([P, K], BF16, tag=f"elT{dc}")
            nc.sync.dma_start_transpose(out=tl[:], in_=el_dram[:, dc * P:(dc + 1) * P])
            elT.append(tl)

        # ---- main loop over 128-row tiles (software-pipelined emission:
        # prep of tile t+1 is emitted before gemm/finalize of tile t so the
        # in-order engine queues never block next-tile prep behind
        # current-tile reductions) ----

        def prep(t):
            """Load z tile, row norms, fp32 PE-transpose + bf16 hi/lo split."""
            zt_ = zpool.tile([P, D], F32, tag="z", name=f"z{t}")
            nc.sync.dma_start(out=zt_[:], in_=z[t * P:(t + 1) * P, :])

            zsq = zpool.tile([P, D], F32, tag="zsq", name=f"zsq{t}")
            nsum = small.tile([P, 1], F32, tag="nsum", name=f"nsum{t}")
            nc.scalar.activation(zsq[:], zt_[:], Square, accum_out=nsum[:])
            nrm = small.tile([P, 1], F32, tag="nrm", name=f"nrm{t}")
            nc.scalar.activation(nrm[:], nsum[:], Sqrt)
            nrmc = small.tile([P, 1], F32, tag="nrmc", name=f"nrmc{t}")
            nc.vector.tensor_scalar_max(nrmc[:], nrm[:], 1e-12)
            inv = small.tile([P, 1], F32, tag="inv", name=f"inv{t}")
            nc.vector.reciprocal(inv[:], nrmc[:])

            # PE-transpose fp32 z chunks, then split hi/lo straight out of
            # PSUM (cast on ACT, exact residual on DVE)
            zhT = tpool.tile([P, N_DC, P], BF16, tag="zhT", name=f"zhT{t}")
            zlT = tpool.tile([P, N_DC, P], BF16, tag="zlT", name=f"zlT{t}")
            for dc in range(N_DC):
                tpf = tppool.tile([P, P], F32, tag="tp", name=f"tp{t}_{dc}")
                nc.tensor.transpose(tpf[:], zt_[:, dc * P:(dc + 1) * P], ident[:])
                nc.scalar.copy(out=zhT[:, dc, :], in_=tpf[:])
                nc.vector.tensor_tensor(
                    zlT[:, dc, :], tpf[:], zhT[:, dc, :], op=sub)
            return dict(z=zt_, inv=inv, nsum=nsum, zhT=zhT, zlT=zlT)

        def gemm(t, st):
            """48 accumulating matmuls -> sim [128, K] in SBUF."""
            zhT, zlT = st["zhT"], st["zlT"]
            sims = [pspool.tile([P, KC], F32, tag="sims", name=f"sims{t}_{kc}")
                    for kc in range(N_KC)]
            for dc in range(N_DC):
                for kc in range(N_KC):
                    ks = slice(kc * KC, (kc + 1) * KC)
                    nc.tensor.matmul(
                        sims[kc][:], zhT[:, dc, :], ehT[dc][:, ks],
                        start=(dc == 0), stop=False)
                    nc.tensor.matmul(
                        sims[kc][:], zhT[:, dc, :], elT[dc][:, ks],
                        start=False, stop=False)
            for dc in range(N_DC):
                for kc in range(N_KC):
                    ks = slice(kc * KC, (kc + 1) * KC)
                    nc.tensor.matmul(
                        sims[kc][:], zlT[:, dc, :], ehT[dc][:, ks],
                        start=False, stop=(dc == N_DC - 1))
            sim_sb = spool.tile([P, K], F32, tag="sim", name=f"sim{t}")
            for kc in range(N_KC):
                nc.scalar.copy(
                    out=sim_sb[:, kc * KC:(kc + 1) * KC], in_=sims[kc][:])
            return sim_sb

        def finalize(t, st, sim_sb):
            """argmax, gather, outputs, commitment partials.

            Commitment uses the closed form
              sum_d (zn - g)^2 = nsum*inv^2 - 2*inv*s_top*||E_idx|| + ||E_idx||^2
            (s_top = max sim = z . Ehat_idx), so nothing wide depends on the
            gathered rows -- only [P,1] ops, which can't clog engine queues.
            """
            rows = slice(t * P, (t + 1) * P)
            mx8 = small.tile([P, 8], F32, tag="mx8", name=f"mx8{t}")
            ix8 = small.tile([P, 8], U32, tag="ix8", name=f"ix8{t}")
            nc.vector.max(mx8[:], sim_sb[:])
            nc.vector.max_index(ix8[:], mx8[:], sim_sb[:])
            nc.sync.dma_start(out=oi[t, :, :], in_=ix8[:, 0:1])

            g = gpool.tile([P, D], F32, tag="g", name=f"g{t}")
            nc.gpsimd.indirect_dma_start(
                out=g[:], out_offset=None, in_=e,
                in_offset=bass.IndirectOffsetOnAxis(ap=ix8[:, 0:1], axis=0))
            nc.sync.dma_start(out=oe[rows, :], in_=g[:])

            en = small.tile([P, 1], F32, tag="en", name=f"en{t}")
            nc.gpsimd.indirect_dma_start(
                out=en[:], out_offset=None, in_=enrm_dram[:],
                in_offset=bass.IndirectOffsetOnAxis(ap=ix8[:, 0:1], axis=0))

            inv = st["inv"]
            a = small.tile([P, 1], F32, tag="ca", name=f"ca{t}")
            nc.vector.tensor_tensor(a[:], inv[:], inv[:], op=mul)       # inv^2
            nc.vector.tensor_tensor(a[:], a[:], st["nsum"][:], op=mul)  # nsum*inv^2
            b = small.tile([P, 1], F32, tag="cb", name=f"cb{t}")
            nc.vector.tensor_tensor(b[:], mx8[:, 0:1], en[:], op=mul)   # s_top*||E||
            nc.vector.tensor_tensor(b[:], b[:], inv[:], op=mul)         # *inv
            nc.vector.tensor_scalar_mul(b[:], b[:], -2.0)               # -2*...
            c2 = small.tile([P, 1], F32, tag="cc", name=f"cc{t}")
            nc.vector.tensor_tensor(c2[:], en[:], en[:], op=mul)        # ||E||^2
            csum = small.tile([P, 1], F32, tag="csum", name=f"csum{t}")
            nc.vector.tensor_tensor(csum[:], a[:], b[:], op=add)
            nc.vector.tensor_tensor(csum[:], csum[:], c2[:], op=add)
            nc.sync.dma_start(out=oc[t, :, :], in_=csum[:])

        state = {0: prep(0)}
        for t in range(n_tiles):
            sim_sb = gemm(t, state[t])
            if t + 1 < n_tiles:
                state[t + 1] = prep(t + 1)
            finalize(t, state[t], sim_sb)
            del state[t]

    nc.compile()
    return nc


def _get_nc(n_rows: int):
    if n_rows not in _CACHE:
        _CACHE[n_rows] = _build_nc(n_rows)
    return _CACHE[n_rows]


def kernel(z: np.ndarray, embeddings: np.ndarray, _trace: bool = False):
    z = np.ascontiguousarray(np.asarray(z, dtype=np.float32))
    emb = np.ascontiguousarray(np.asarray(embeddings, dtype=np.float32))
    assert z.shape == (N, D) and emb.shape == (K, D)
    ns = N // N_CORES

    nc = _get_nc(ns)
    in_maps = [
        {"z": z[c * ns:(c + 1) * ns], "emb": emb} for c in range(N_CORES)
    ]
    if _trace:
        _ensure_ntff_hook()
    res = run_bass_kernel_spmd(
        nc, in_maps, core_ids=list(range(N_CORES)), trace=_trace)

    out_emb = np.concatenate([r["out_emb"] for r in res.results], axis=0)
    idx = np.concatenate(
        [r["out_idx"].reshape(-1) for r in res.results]).astype(np.int32)
    csum = np.concatenate(
        [r["out_csum"].reshape(-1) for r in res.results])

    commitment_loss = np.float32(0.25 * (csum.astype(np.float64).sum() / (N * D)))
    counts = np.bincount(idx, minlength=K).astype(np.float64)
    avg = counts / N
    perplexity = np.float32(np.exp(-np.sum(avg * np.log(avg + 1e-10))))

    if _trace:
        kernel.last_exec_time_ns = res.exec_time_ns
        kernel.last_res = res
    return out_emb, idx, commitment_loss, perplexity
